# revision 14
# baseline (speedup 1.0000x reference)
"""NonLocalConvBlock Trainium2 kernel (8-core SPMD, row-sharded flash-softmax).

out = Wo @ softmax_ic( softmax_row(theta @ phi) @ g ) + bo + x   (with the
torch-reshape semantics of the reference: theta/g are flat row-major views).

Sharding: rows of theta (N=9216) split 1152/core; phi/g replicated.
Per core: sT tiles [m=128, r=1152] on PSUM (bf16 matmul) -> exp per m-tile
(split ACT/DVE, Schraudolph approx on DVE) -> accumulate yT = g_aug^T @ exp
(ones column gives row-sums) -> transpose -> softmax over ic -> partial
output conv with this core's 4-channel slice of Wo. No collective: each core
returns its partial [C, N] conv output (+ (x + bo)/8 so the host-side sum of
the 8 partials reconstructs Wo@y + bo + x exactly).

HAM note: the PE clock sits at 1.2 GHz unless the activity monitor sees
high-row-activity matmuls (measured: contraction-32 matmuls NEVER warm the
clock to 2.4 GHz, contraction-128 ones do). So every main-loop matmul is
built to use all 128 PE rows:
  - mm1 stationary is phi4 = phi/4 replicated on 4x32 partitions (built for
    free by the 1x1-conv projection with vertically doubled weights), moving
    is thT4 = theta^T stacked 4x on partitions; the 4 block-sums reproduce
    theta@phi exactly (the /4 cancels the replication).
  - mm2 already contracts over 128 pixels (stationary g tile [128, 33]).
  - a burst of dense 128x128 warm-up matmuls runs while x streams in, so
    the projections and the loop run warm from the start.

Main loop is software-pipelined (mm1(i+1) emitted before mm2(i)) so the PE
works through mm2(i) + mm1(i+2) while ACT/DVE run exp(i+1).

PSUM layout (8 banks x 2KB):
  banks 0-5: sp0/sp1/sp2 [128, 1024]: triple-buffered score tiles (first
             1024 of each 1152-wide r-block)
  bank  6:   combo [128, 512]: cols 0:384 = the last-128 r-columns of the
             three score buffers; cols 384:512 (partitions 0:33) = yacc2
  bank  7:   yacc01 [128, 512]: partitions 0:33 = yacc0, 64:97 = yacc1
             (stacked matmul accumulators at different partition offsets)
"""

import numpy as np
import ml_dtypes

import concourse.bacc as bacc
import concourse.bass as bass
import concourse.mybir as mybir
from concourse.tile import TileContext
from concourse.bass_utils import run_bass_kernel_spmd

F32 = mybir.dt.float32
BF16 = mybir.dt.bfloat16
I16 = mybir.dt.int16
AFT = mybir.ActivationFunctionType
ALU = mybir.AluOpType
AX = mybir.AxisListType

B, C, H, W = 1, 64, 96, 96
IC = C // 2            # 32
N = B * H * W          # 9216
NCORES = 8
NL = N // NCORES       # 1152 rows per core
CH_L = IC // NCORES    # 4 local proj_t channels per core
RT = NL // 128         # 9 row tiles
MT = N // 128          # 72 col (m) tiles
NW = 164               # W2 cols: [phi4 0:128 | pg 128:160 | pt 160:164]

# fixed r-chunking for the yT accumulation (bank-sized accumulators)
Y_CHUNKS = [(0, 512), (512, 512), (1024, 128)]

# Schraudolph fast-exp constants: bitcast((int32)(A*x + B)) ~= exp(x).
# We emit bf16 directly: int16 bits = round((A*x + B) / 2^16), one
# tensor_scalar per region (max rel err ~3%, fine at the 2e-2 gate).
EXP_A16 = float(1 << 23) / float(np.log(2.0)) / 65536.0
EXP_B16 = (127.0 * (1 << 23) - 486411.0) / 65536.0
# m-tile -> exp engine: 0=ACT (exact), 1=DVE (approx), 2=Pool (approx)
EXPSPLIT = True
DEBUG_TAPS = False
N_WARM_MM = 28         # dense 128x128 PE warm-up matmuls during input DMA


def _mk_exp_schedule():
    # largest-remainder interleave, shares ~ inverse modeled cost
    shares = (36.0, 36.0, 0.0)
    acc = [0.0, 0.0, 0.0]
    out = []
    for _ in range(MT):
        for i in range(3):
            acc[i] += shares[i] / MT
        pick = max(range(3), key=lambda i: acc[i])
        acc[pick] -= 1.0
        out.append(pick)
    return out


_EXP_SCHED = _mk_exp_schedule()


def _exp_engine(mi):
    if not EXPSPLIT:
        return 0
    return _EXP_SCHED[mi]


def build():
    nc = bacc.Bacc(None, target_bir_lowering=False, debug=False)

    x_cm = nc.dram_tensor("x_cm", [C, N], BF16, kind="ExternalInput")
    W2 = nc.dram_tensor("W2", [128, NW], BF16, kind="ExternalInput")
    b_all = nc.dram_tensor("b_all", [NW, 1], F32, kind="ExternalInput")
    bo8 = nc.dram_tensor("bo8", [C, 1], F32, kind="ExternalInput")
    WoTl = nc.dram_tensor("WoTl", [CH_L, C], BF16, kind="ExternalInput")
    eye128 = nc.dram_tensor("eye128", [128, 128], BF16, kind="ExternalInput")
    eye33 = nc.dram_tensor("eye33", [IC + 1, IC + 1], F32, kind="ExternalInput")
    eye128f = nc.dram_tensor("eye128f", [128, 128], F32, kind="ExternalInput")
    out_d = nc.dram_tensor("out", [C, N], BF16, kind="ExternalOutput")
    if DEBUG_TAPS:
        tap_phi4 = nc.dram_tensor("tap_phi4", [128, N], BF16, kind="ExternalOutput")
        tap_pgt = nc.dram_tensor("tap_pgt", [36, N], BF16, kind="ExternalOutput")
        tap_thT4 = nc.dram_tensor("tap_thT4", [128, NL], BF16, kind="ExternalOutput")
        tap_g = nc.dram_tensor("tap_g", [128, MT * (IC + 1)], BF16, kind="ExternalOutput")
        tap_yT = nc.dram_tensor("tap_yT", [IC + 1, NL], F32, kind="ExternalOutput")

    with TileContext(nc) as tc:
        with (
            tc.tile_pool(name="dram", bufs=1, space="DRAM") as dpool,
            tc.tile_pool(name="sb", bufs=1) as pool,
        ):
            pt_flat = dpool.tile([CH_L, N], BF16)   # proj_t local, flat
            pgA_flat = dpool.tile([8, N], BF16)     # proj_g channels 0:8
            pgB_flat = dpool.tile([IC - 8, N], BF16)  # proj_g channels 8:32
            y_loc = dpool.tile([NL, IC], BF16)      # this core's y rows

            # ---- load inputs (eye first so PE warm-up can start, then
            # weights, then x twice into the 128-partition stack) ----
            # queue plan: sync = eye (warm-up gate) then x half-0,
            # scalar = x half-1 (both x halves stream in parallel),
            # gpsimd = W2 + biases + late smalls
            eye_sb = pool.tile([128, 128], BF16)
            nc.sync.dma_start(eye_sb[:], eye128[:])
            wa_sb = pool.tile([128, NW], BF16)
            nc.gpsimd.dma_start(wa_sb[:], W2[:])
            baP_sb = pool.tile([128, 1], F32)
            nc.scalar.dma_start(baP_sb[:], b_all[0:128, :])
            baG_sb = pool.tile([36, 1], F32)
            nc.scalar.dma_start(baG_sb[:], b_all[128:NW, :])
            x2_sb = pool.tile([128, N], BF16)
            # both partition-halves of the doubled x stack stream in parallel
            # across the three hwdge queues (sync/scalar take the front half
            # of the pixel range for each partition half, gpsimd the back)
            for xc in range(2):
                xsl = slice(xc * 2304, (xc + 1) * 2304)
                nc.sync.dma_start(x2_sb[0:C, xsl], x_cm[:, xsl])
                nc.scalar.dma_start(x2_sb[C:128, xsl], x_cm[:, xsl])
            for xc in range(2):
                xsl = slice(4608 + xc * 2304, 4608 + (xc + 1) * 2304)
                nc.gpsimd.dma_start(x2_sb[0:C, xsl], x_cm[:, xsl])
                nc.gpsimd.dma_start(x2_sb[C:128, xsl], x_cm[:, xsl])
            eyef_sb = pool.tile([IC + 1, IC + 1], F32)
            nc.gpsimd.dma_start(eyef_sb[:], eye33[:])
            eyeff_sb = pool.tile([128, 128], F32)
            nc.gpsimd.dma_start(eyeff_sb[:], eye128f[:])
            bo8_sb = pool.tile([C, 1], F32)
            nc.gpsimd.dma_start(bo8_sb[:], bo8[:])
            wo_sb = pool.tile([CH_L, C], BF16)
            nc.gpsimd.dma_start(wo_sb[:], WoTl[:])

            # prewarm the ACT exp table while inputs stream in
            warm = pool.tile([1, 1], F32)
            nc.scalar.activation(warm[:], baP_sb[0:1, 0:1], AFT.Exp)

            # ---- fused 1x1 projections ----
            # pass A: [pg; pt] (36 partitions); pass B: phi4 (128 partitions,
            # = phi/4 replicated 4x). Both stationaries span all 128 PE rows
            # (weights vertically doubled, x stacked twice) to keep HAM busy.
            phi4_sb = pool.tile([128, N], BF16)
            pgt_sb = pool.tile([36, N], BF16)
            with tc.tile_pool(name="pp", bufs=6, space="PSUM") as pp:
                # dense warm-up: unthrottle the PE clock while x streams in
                # (just enough to cover the x-DMA head; more would delay
                # pass A on the serial PE queue)
                wrm = pp.tile([128, 128], F32, tag="wrm", bufs=1, name="wrm")
                for _ in range(N_WARM_MM):
                    nc.tensor.matmul(
                        wrm[:, :], eye_sb[:], eye_sb[:], start=True, stop=True
                    )
                # pass A first so the pt/pg DRAM round-trips (theta/g
                # regroup, long scattered DMAs) overlap pass B + loop head
                ptws = []
                for k in range(N // 512):
                    sl = slice(k * 512, (k + 1) * 512)
                    psA = pp.tile([36, 512], F32, tag="projA", name="psA", bufs=3)
                    nc.tensor.matmul(
                        psA[:], wa_sb[:, 128:NW], x2_sb[:, sl],
                        start=True, stop=True,
                    )
                    nc.scalar.activation(
                        pgt_sb[:, sl], psA[:, :],
                        AFT.Identity, bias=baG_sb[:, :],
                    )
                    if k % 6 == 5:
                        wsl = slice((k - 5) * 512, (k + 1) * 512)
                        ptws.append(
                            nc.sync.dma_start(pt_flat[:, wsl], pgt_sb[32:36, wsl])
                        )
                pgwA = nc.sync.dma_start(pgA_flat[:], pgt_sb[0:8, :])
                pgwB = nc.sync.dma_start(pgB_flat[:], pgt_sb[8:32, :])
                for k in range(N // 512):
                    sl = slice(k * 512, (k + 1) * 512)
                    psB = pp.tile([128, 512], F32, tag="projB", name="psB", bufs=3)
                    nc.tensor.matmul(
                        psB[:], wa_sb[:, 0:128], x2_sb[:, sl],
                        start=True, stop=True,
                    )
                    eng = (nc.vector, nc.vector, nc.scalar)[k % 3]
                    if eng is nc.scalar:
                        nc.scalar.activation(
                            phi4_sb[:, sl], psB[:, :],
                            AFT.Identity, bias=baP_sb[:, :],
                        )
                    else:
                        eng.tensor_scalar(
                            phi4_sb[:, sl], psB[:, :], baP_sb[:, :],
                            None, ALU.add,
                        )
            phi4 = phi4_sb[:, :]

            # ---- flat-view reshapes back in (torch reshape semantics) ----
            # theta rows for this core: th_rows[r, j] = pt_flat.flat[32r + j];
            # read rows contiguously, broadcast 4x along partitions' free dim,
            # then PE-transpose each [128, 128] tile to thT4 [4x32, 1152].
            th_sb = pool.tile([128, RT, 1, IC], BF16)
            pt_lin = pt_flat[:].rearrange("c q -> (c q)")
            th_src = pt_lin.rearrange("(rt p j) -> p rt j", p=128, j=IC)
            thr = nc.sync.dma_start(th_sb[:, 0:5, 0, :], th_src[:, 0:5, :])
            thr2 = nc.scalar.dma_start(th_sb[:, 5:RT, 0, :], th_src[:, 5:RT, :])
            for _ptw in ptws:
                bass._add_dep_helper(thr.ins, _ptw.ins, sync=True, reason="th after ptw")
                bass._add_dep_helper(thr2.ins, _ptw.ins, sync=True, reason="th2 after ptw")

            # g rows [9216, 32] -> SBUF [128, mt, 33] with ones in col 32;
            # m-tiles 0:18 come from pg channels 0:8 (pgA), 18:72 from pgB,
            # so each flat-view read pairs with exactly one DMA write
            th4_sb = pool.tile([128, RT, 4, IC], BF16)
            nc.vector.tensor_copy(
                th4_sb[:], th_sb[:].broadcast_to([128, RT, 4, IC])
            )
            g_sb = pool.tile([128, MT, IC + 1], BF16)
            gA_src = pgA_flat[:].rearrange("i q -> (i q)").rearrange(
                "(mt p j) -> p mt j", p=128, j=IC
            )
            gB_src = pgB_flat[:].rearrange("i q -> (i q)").rearrange(
                "(mt p j) -> p mt j", p=128, j=IC
            )
            thT4 = pool.tile([128, NL], BF16)
            with tc.tile_pool(name="tp", bufs=3, space="PSUM") as tpp:
                for rt in range(RT):
                    tps = tpp.tile([128, 128], BF16, tag="tt")
                    nc.tensor.transpose(
                        tps[:],
                        th4_sb[:, rt, :, :].rearrange("p a b -> p (a b)"),
                        eye_sb[:, :],
                    )
                    eng = (nc.vector, nc.scalar)[rt % 2]
                    if eng is nc.scalar:
                        nc.scalar.activation(
                            thT4[:, rt * 128 : (rt + 1) * 128], tps[:], AFT.Copy
                        )
                    else:
                        eng.tensor_copy(thT4[:, rt * 128 : (rt + 1) * 128], tps[:])
            grA = nc.sync.dma_start(g_sb[:, 0:18, 0:IC], gA_src[:, :, :])
            bass._add_dep_helper(grA.ins, pgwA.ins, sync=True, reason="gA after pgwA")
            grB = nc.sync.dma_start(g_sb[:, 18:MT, 0:IC], gB_src[:, :, :])
            bass._add_dep_helper(grB.ins, pgwB.ins, sync=True, reason="gB after pgwB")
            nc.vector.memset(g_sb[:, :, IC : IC + 1], 1.0)

            # residual staging: ob = x/8 + bo/8, filled in during the loop
            ob = pool.tile([C, N], F32)
            out_sb = pool.tile([C, N], BF16)

            # ---- main fused loop over 72 m-tiles (software-pipelined) ----
            with (
                tc.tile_pool(name="acc", bufs=1, space="PSUM") as accp,
                tc.tile_pool(name="spp", bufs=1, space="PSUM") as spp,
            ):
                sps = [
                    spp.tile([128, 1024], F32, name=f"sp{i}") for i in range(3)
                ]
                combo = spp.tile([128, 512], F32, name="combo")
                yacc01 = accp.tile([128, 512], F32, name="yacc01")
                # accumulator views: r 0:512, 512:1024 (stacked), 1024:1152
                yview = [
                    yacc01[0 : IC + 1, :],
                    yacc01[64 : 64 + IC + 1, :],
                    combo[0 : IC + 1, 384:512],
                ]
                ets = {}

                def mm1(mi):
                    msl = slice(mi * 128, (mi + 1) * 128)
                    big = sps[mi % 3]
                    soff = (mi % 3) * 128
                    for off, w in ((0, 512), (512, 512)):
                        nc.tensor.matmul(
                            big[:, off : off + w],
                            phi4[:, msl],
                            thT4[:, off : off + w],
                            start=True,
                            stop=True,
                        )
                    nc.tensor.matmul(
                        combo[:, soff : soff + 128],
                        phi4[:, msl],
                        thT4[:, 1024:1152],
                        start=True,
                        stop=True,
                    )

                def do_exp(mi):
                    # whole-tile main engine alternates ACT/DVE per tile;
                    # the 128-col combo tail goes to the OTHER engine so the
                    # shared combo bank is freed fast (mm1(mi+3) reuses it)
                    big = sps[mi % 3]
                    soff = (mi % 3) * 128
                    et = pool.tile([128, NL], BF16, tag="exp", bufs=5, name="et")
                    ets[mi] = et
                    if mi % 2 == 0:
                        nc.vector.tensor_scalar(
                            et[:, 1024:1152].bitcast(I16),
                            combo[:, soff : soff + 128],
                            EXP_A16, EXP_B16, ALU.mult, ALU.add,
                        )
                        nc.scalar.activation(et[:, 0:1024], big[:, :], AFT.Exp)
                    else:
                        nc.scalar.activation(
                            et[:, 1024:1152], combo[:, soff : soff + 128], AFT.Exp
                        )
                        nc.vector.tensor_scalar(
                            et[:, 0:1024].bitcast(I16), big[:, :],
                            EXP_A16, EXP_B16, ALU.mult, ALU.add,
                        )

                def mm2(mi):
                    et = ets.pop(mi)
                    for (off, w), yp in zip(Y_CHUNKS, yview):
                        nc.tensor.matmul(
                            yp,
                            g_sb[:, mi, :],
                            et[:, off : off + w],
                            start=(mi == 0),
                            stop=(mi == MT - 1),
                            skip_group_check=True,
                        )

                for mi in range(MT):
                    mm1(mi)
                    if mi > 0:
                        mm2(mi - 1)
                    do_exp(mi)
                    # fill the residual staging in spare engine cycles
                    if mi % 4 == 2:
                        k = mi // 4
                        sl = slice(k * 512, (k + 1) * 512)
                        nc.gpsimd.tensor_scalar(
                            ob[:, sl], x2_sb[0:C, sl],
                            1.0 / NCORES, bo8_sb[:, :], ALU.mult, ALU.add,
                        )
                mm2(MT - 1)
                yT_sb = pool.tile([IC + 1, NL], F32)
                for i, ((off, w), yp) in enumerate(zip(Y_CHUNKS, yview)):
                    eng = (nc.vector, nc.scalar, nc.vector)[i]
                    if eng is nc.scalar:
                        nc.scalar.activation(
                            yT_sb[:, off : off + w], yp, AFT.Copy
                        )
                    else:
                        eng.tensor_copy(yT_sb[:, off : off + w], yp)

            # ---- per-row scale by 1/rowsum, softmax over ic ----
            ysc = pool.tile([128, RT, IC], F32)
            y_fin = pool.tile([128, RT, IC], BF16)
            with tc.tile_pool(name="fp", bufs=2, space="PSUM") as fpp:
                dumf = fpp.tile([IC, 128], F32, tag="dumf", bufs=1, name="dumf")
                dumb = fpp.tile([IC, 128], BF16, tag="dumb", bufs=1, name="dumb")

                def pe_warm(in_):
                    # keep the PE clock unthrottled through the softmax phase;
                    # the input ties each dummy to the preceding stage so the
                    # scheduler cannot hoist it out of the gap
                    if in_.dtype == F32:
                        nc.tensor.transpose(dumf[:], in_, eyeff_sb[:, :])
                    else:
                        nc.tensor.transpose(dumb[:], in_, eye_sb[:, :])

                ey = pool.tile([128, RT, IC], F32)
                sm = pool.tile([128, RT, 1], F32)
                rsm = pool.tile([128, RT, 1], F32)
                y_dst = y_loc[:].rearrange("(rt p) j -> p rt j", p=128)
                ylws = []
                for h0, h1, wq in ((0, 5, nc.sync), (5, RT, nc.scalar)):
                    for rt in range(h0, h1):
                        ytp = fpp.tile(
                            [128, IC + 1], F32, tag="yt", bufs=4, name="ytrans"
                        )
                        nc.tensor.transpose(
                            ytp[:],
                            yT_sb[:, rt * 128 : (rt + 1) * 128],
                            eyef_sb[:, :],
                        )
                        # fused: ey = exp(ytp * (1/rowsum)), rowsum accum on ACT
                        rs = pool.tile([128, 1], F32, tag="rs", bufs=4, name="rs")
                        nc.vector.reciprocal(rs[:], ytp[:, IC : IC + 1])
                        nc.scalar.activation(
                            ey[:, rt, :], ytp[:, 0:IC], AFT.Exp,
                            scale=rs[:], accum_out=sm[:, rt, :],
                        )
                        if rt % 3 == 2:
                            pe_warm(ey[:, rt, :])
                    hsl = slice(h0, h1)
                    nc.vector.reciprocal(
                        rsm[:, hsl, :].rearrange("p a b -> p (a b)"),
                        sm[:, hsl, :].rearrange("p a b -> p (a b)"),
                    )
                    nc.vector.tensor_tensor(
                        y_fin[:, hsl, :], ey[:, hsl, :],
                        rsm[:, hsl, :].broadcast_to([128, h1 - h0, IC]),
                        ALU.mult,
                    )
                    pe_warm(y_fin[:, h0, :])
                    # ---- y rows -> DRAM (this half) ----
                    ylws.append(wq.dma_start(y_dst[:, hsl, :], y_fin[:, hsl, :]))
                ylw, ylwb = ylws
                # spurious readbacks whose completion falls inside the DMA
                # window; the dummies chained to them keep the PE warm there
                spr = nc.scalar.dma_start(th_sb[:, 0, 0, :], y_loc[0:128, :])
                bass._add_dep_helper(spr.ins, ylw.ins, sync=True, reason="spur after ylw")
                pe_warm(th_sb[:, 0, 0, :])
                yimg = pool.tile([CH_L, N], BF16)
                y_img_src = y_loc[:].rearrange("(c q) j -> c (q j)", c=CH_L)
                yir = nc.gpsimd.dma_start(yimg[0:2, :], y_img_src[0:2, :])
                bass._add_dep_helper(yir.ins, ylw.ins, sync=True, reason="yimgA after ylwA")
                yir2 = nc.gpsimd.dma_start(yimg[2:CH_L, :], y_img_src[2:CH_L, :])
                bass._add_dep_helper(yir2.ins, ylw.ins, sync=True, reason="yimgB after ylwA")
                bass._add_dep_helper(yir2.ins, ylwb.ins, sync=True, reason="yimgB after ylwB")
                spr3 = nc.scalar.dma_start(th_sb[:, 2, 0, :], y_loc[256:384, :])
                bass._add_dep_helper(spr3.ins, yir.ins, sync=True, reason="spur3 after yimgA")
                pe_warm(th_sb[:, 2, 0, :])


            if DEBUG_TAPS:
                nc.sync.dma_start(tap_phi4[:], phi4_sb[:, :])
                nc.sync.dma_start(tap_pgt[:], pgt_sb[:, :])
                nc.sync.dma_start(tap_thT4[:], thT4[:])
                nc.sync.dma_start(
                    tap_g[:], g_sb[:].rearrange("p a b -> p (a b)")
                )
                nc.sync.dma_start(tap_yT[:], yT_sb[:])

            # ---- partial out conv ----
            tok = pool.tile([1, 8], F32, name="tok")
            wqs = [nc.sync, nc.gpsimd]
            with tc.tile_pool(name="op", bufs=4, space="PSUM") as opp:
                for k in range(N // 512):
                    sl = slice(k * 512, (k + 1) * 512)
                    pso = opp.tile([C, 512], F32, tag="o", name="otile")
                    nc.tensor.matmul(
                        pso[:], wo_sb[:], yimg[:, sl], start=True, stop=True
                    )
                    nc.vector.tensor_tensor(
                        out_sb[:, sl], pso[:], ob[:, sl], ALU.add
                    )
                    if k % 2 == 1:
                        wsl = slice((k - 1) * 512, (k + 1) * 512)
                        wqs[(k // 2) % 2].dma_start(out_d[:, wsl], out_sb[:, wsl])

    nc.compile()
    return nc


_NC = None


def kernel(**inputs):
    global _NC
    x = np.ascontiguousarray(np.asarray(inputs["x"], dtype=np.float32))
    Wt = np.asarray(inputs["Wt"], dtype=np.float32)
    bt = np.asarray(inputs["bt"], dtype=np.float32)
    Wp = np.asarray(inputs["Wp"], dtype=np.float32)
    bp_ = np.asarray(inputs["bp"], dtype=np.float32)
    Wg = np.asarray(inputs["Wg"], dtype=np.float32)
    bg_ = np.asarray(inputs["bg"], dtype=np.float32)
    Wo = np.asarray(inputs["Wo"], dtype=np.float32)
    bo_ = np.asarray(inputs["bo"], dtype=np.float32)

    if _NC is None:
        _NC = build()

    X = x.reshape(C, N)
    eye = np.eye(128, dtype=ml_dtypes.bfloat16)
    common = {
        "x_cm": X.astype(ml_dtypes.bfloat16),
        "bo8": (bo_ / float(NCORES)).reshape(C, 1).astype(np.float32),
        "eye128": eye,
        "eye33": np.eye(IC + 1, dtype=np.float32),
        "eye128f": np.eye(128, dtype=np.float32),
    }
    in_maps = []
    for d in range(NCORES):
        m = dict(common)
        wt_l = Wt[d * CH_L : (d + 1) * CH_L, :]
        bt_l = bt[d * CH_L : (d + 1) * CH_L]
        # W2[p, f<128] = Wp[f%32, p%64]/8 : phi4 = phi/4 off the doubled-x
        # stack (the /4 cancels mm1's 4-block replication sum).
        blockP = np.tile(Wp.T / 8.0, (2, 4))            # [128, 128]
        blockG = np.tile(Wg.T / 2.0, (2, 1))            # [128, 32]
        blockT = np.tile(wt_l.T / 2.0, (2, 1))          # [128, 4]
        m["W2"] = np.ascontiguousarray(
            np.concatenate([blockP, blockG, blockT], axis=1)
        ).astype(ml_dtypes.bfloat16)
        m["b_all"] = np.concatenate(
            [np.tile(bp_ / 4.0, 4), bg_, bt_l]
        ).reshape(NW, 1).astype(np.float32)
        m["WoTl"] = np.ascontiguousarray(
            Wo[:, d * CH_L : (d + 1) * CH_L].T
        ).astype(ml_dtypes.bfloat16)
        in_maps.append(m)

    global _last_in_maps
    _last_in_maps = in_maps
    res = run_bass_kernel_spmd(_NC, in_maps, list(range(NCORES)))
    out = np.zeros((C, N), dtype=np.float32)
    for d in range(NCORES):
        out += np.asarray(res.results[d]["out"], dtype=np.float32)
    return out.reshape(B, C, H, W)


_last_in_maps = None
